# revision 3
# baseline (speedup 1.0000x reference)
"""Trainium2 Bass kernel for nn_CurveGraphic2d (retrieval_knn), v2.

Computes, for B=16 cubic Bezier curves, a 256x256 canvas per curve:
    canvas = clip(1 - (min_dist_to_32_samples / w + eps)^aa, 0, 1)

v2 strategy (job-pool sharding, 3 curve-pieces per core):
  * Host: evaluate the 32 samples per curve; emit one "job" per active
    pixel column x and y-tile: the samples relevant to that column
    (|sx - x| <= margin, margin = w + 0.6 -- pixels farther than w
    render 0, so wide fixed margins are wasted work).  Jobs with more
    than M_CAP samples split into sub-jobs (host merges with min).
  * Curves split into 24 round-robin pieces (3 slots x 8 cores), more
    pieces for heavier curves; pieces of similar profile share a slot,
    so the rank-wise-max slot schedule pads little.
  * Device: one DMA brings phi+psi tables; fp32r matmuls phi^T @ psi
    produce T[p, col] = squared distance from pixel row p to the col's
    sample.  Single-sample columns are written by the matmul directly
    into the strip (no reduction needed); multi-sample jobs go through
    grouped strided tensor_reduce mins (tensor_tensor min for M=2).
    Per slot the ACT engine runs Relu -> Ln(x/w^2) -> Exp(aa/2 * .) in
    PSUM and emits bf16; DMAs stream the three slot strips out.
  * Host: p -> clip(1 - p, 0, 1) and scatter/min-merge columns into the
    canvases (placement + unshard).
"""

import math

import numpy as np

H, W = 256, 256
NUM_SAMPLES = 32
MAX_LENGTH = 300.0
EPSILON = 1e-6
N_CORES = 8
SLOTS = 3
MARGIN_PAD = 0.6
PAD_SY = 1500.0
CHUNK_CAP = 512
M_CAP = 4

# DVE cost model for the grouping DP (ns)
RED_FIXED = 250.0
RED_PER_EL = 1.04


# ----------------------------------------------------------------------------
# Host-side geometry (mirrors reference.py in float64)
# ----------------------------------------------------------------------------

def _bezier_eval(cp, ts):
    K = cp.shape[0]
    n = K - 1
    i = np.arange(K)
    binom = np.array([math.comb(n, k) for k in range(K)], dtype=np.float64)
    t = ts[:, None]
    basis = binom * (t ** i) * ((1.0 - t) ** (n - i))
    return basis @ cp


def _decasteljau_left(cp, t):
    pts = cp.copy()
    left = [cp[0]]
    for _ in range(cp.shape[0] - 1):
        pts = (1.0 - t) * pts[:-1] + t * pts[1:]
        left.append(pts[0])
    return np.stack(left)


def compute_samples(inputs):
    """[B, K, 2] normalized control points -> [B, S, 2] sample points (y, x)."""
    ts = np.linspace(0.0, 1.0, NUM_SAMPLES)
    out = []
    for b in range(inputs.shape[0]):
        cp = inputs[b].astype(np.float64) * np.array([H, W], dtype=np.float64)
        approx = _bezier_eval(cp, ts)
        seg = np.diff(approx, axis=0)
        arc = np.sqrt((seg ** 2).sum(-1)).sum()
        t_tr = min(1.0, MAX_LENGTH / (arc + EPSILON))
        out.append(_bezier_eval(_decasteljau_left(cp, t_tr), ts))
    return np.stack(out)  # [B, S, 2] float64


def q11(x):
    """Round to 11 significant bits (safely exact under fp32r's ~12-bit
    input truncation)."""
    x = np.asarray(x, dtype=np.float64)
    m, e = np.frexp(x)
    return np.ldexp(np.round(m * 2048.0), e - 11)


# ----------------------------------------------------------------------------
# Planner
# ----------------------------------------------------------------------------

class Job:
    __slots__ = ("x", "ytile", "rows")

    def __init__(self, x, ytile, rows):
        self.x = x          # pixel column
        self.ytile = ytile  # 0 or 1
        self.rows = rows    # [(sy, sx), ...] float64


def plan_curve(samples, margin):
    """samples [S, 2] (y, x) -> list of Job (single-column windows),
    jobs larger than M_CAP split into balanced sub-jobs."""
    sy = samples[:, 0]
    sx = samples[:, 1]
    lo = np.maximum(np.floor(sx - margin).astype(int), 0)
    hi = np.minimum(np.ceil(sx + margin).astype(int), W - 1)
    active = np.zeros(W, dtype=bool)
    for a, b in zip(lo, hi):
        if a <= b:
            active[a:b + 1] = True
    xs = np.nonzero(active)[0]
    jobs = []
    for x in xs:
        selx = np.abs(sx - x) <= margin
        for yt in (0, 1):
            y0, y1 = yt * 128, yt * 128 + 128
            sely = (sy + margin >= y0) & (sy - margin < y1)
            sel = selx & sely
            n = int(sel.sum())
            if n == 0:
                continue
            rows = list(zip(sy[sel], sx[sel]))
            parts = -(-n // M_CAP)
            for i in range(parts):
                jobs.append(Job(int(x), yt, rows[i::parts]))
    return jobs


class Piece:
    __slots__ = ("curve", "jobs", "m1")

    def __init__(self, curve, jobs):
        # multi-sample jobs (desc by size) and single-sample jobs
        self.jobs = [j for j in jobs if len(j.rows) > 1]
        self.m1 = [j for j in jobs if len(j.rows) == 1]
        self.curve = curve


def make_pieces(all_jobs):
    """all_jobs: per-curve job list -> [SLOTS][N_CORES] pieces.

    Split curve c into k_c round-robin pieces (identical rank profiles),
    k_c proportional to its load; pack pieces into slots longest-profile
    first so each slot's octet holds pieces of similar length."""
    ncell = SLOTS * N_CORES
    sorted_jobs = [sorted(jl, key=lambda j: len(j.rows), reverse=True)
                   for jl in all_jobs]
    loads = [sum(len(j.rows) for j in jl) for jl in sorted_jobs]
    target = sum(loads) / ncell
    k = [max(1, min(N_CORES, int(round(L / target)))) for L in loads]
    while sum(k) > ncell:
        i = min((i for i in range(len(k)) if k[i] > 1),
                key=lambda i: loads[i] / (k[i] - 1))
        k[i] -= 1
    while sum(k) < ncell:
        i = max((i for i in range(len(k)) if k[i] < N_CORES),
                key=lambda i: loads[i] / (k[i] + 1))
        k[i] += 1
    groups = []
    for c, jl in enumerate(sorted_jobs):
        ps = [Piece(c, jl[i::k[c]]) for i in range(k[c])]
        groups.append((len(ps[0].jobs), ps))
    # shortest profiles first: slot 0 small so its tail chain starts
    # earliest, larger slots follow while the ACT engine stays busy
    groups.sort(key=lambda t: t[0])
    flat = [p for _, ps in groups for p in ps]
    return [flat[s * N_CORES:(s + 1) * N_CORES] for s in range(SLOTS)]


def slot_schedule(pieces):
    """Rank-wise max of the pieces' descending multi-job-size lists."""
    ls = [[len(j.rows) for j in p.jobs] for p in pieces]
    n = max((len(x) for x in ls), default=0)
    return [max(2, max((x[i] if i < len(x) else 0) for x in ls))
            for i in range(n)]


def opt_groups(sched):
    """DP: partition the desc-sorted schedule into groups, each padded to
    its max M, minimizing RED_FIXED per group + RED_PER_EL per element."""
    arr = sorted(sched, reverse=True)
    N = len(arr)
    from functools import lru_cache

    @lru_cache(None)
    def dp(i):
        if i >= N:
            return (0.0, ())
        best = (1e30, ())
        top = arr[i]
        for j in range(i + 1, N + 1):
            cost = RED_FIXED + RED_PER_EL * top * (j - i)
            rest, parts = dp(j)
            if cost + rest < best[0]:
                best = (cost + rest, ((j - i, top),) + parts)
        return best

    return list(dp(0)[1])  # [(count, M)]


def pack_chunks(slot_groups):
    """Pack the slots' reduce groups (slot order) into <=CHUNK_CAP-col
    PSUM chunks; groups may split at chunk boundaries.  Chunk spans are
    rounded up to even (fp32r matmul requires even moving/dst widths).
    Returns (chunks, reduces):
      chunks:  [total cols per chunk, even]
      reduces: [(chunk_idx, chunk_col, slot, red_rank, g, M)]
    """
    chunks = []
    reduces = []
    cur = 0
    ranks = [0] * SLOTS
    for s, groups in enumerate(slot_groups):
        for g, M in groups:
            while g > 0:
                if not chunks or cur + M > CHUNK_CAP - 1:
                    chunks.append(0)
                    cur = 0
                take = min(g, (CHUNK_CAP - 1 - cur) // M)
                reduces.append((len(chunks) - 1, cur, s, ranks[s], take, M))
                cur += take * M
                chunks[-1] = cur + (cur % 2)  # pad col if odd
                ranks[s] += take
                g -= take
    return chunks, reduces


class Plan:
    pass


def plan_all(inputs, widths, aas):
    B = inputs.shape[0]
    samples = compute_samples(inputs)
    all_jobs = [plan_curve(samples[b], float(widths[b]) + MARGIN_PAD)
                for b in range(B)]
    slots = make_pieces(all_jobs)          # [SLOTS][N_CORES] pieces
    scheds = [slot_schedule(slots[s]) for s in range(SLOTS)]
    groups = [opt_groups(scheds[s]) for s in range(SLOTS)]
    rank_m = [[m for g, m in groups[s] for _ in range(g)]
              for s in range(SLOTS)]
    chunks, reduces = pack_chunks(groups)
    plan = Plan()
    plan.samples = samples
    plan.widths = widths
    plan.aas = aas
    plan.slots = slots
    plan.scheds = scheds
    plan.groups = groups
    plan.rank_m = rank_m
    plan.chunks = chunks
    plan.reduces = reduces
    # even widths: fp32r matmul requires even moving/dst column counts
    plan.m1_len = [-(-max(len(p.m1) for p in slots[s]) // 2) * 2
                   for s in range(SLOTS)]
    plan.red_len = [len(rank_m[s]) for s in range(SLOTS)]
    plan.slot_len = [plan.m1_len[s] + plan.red_len[s] for s in range(SLOTS)]
    plan.chunk_cols = sum(chunks)
    plan.tot_cols = plan.chunk_cols + sum(plan.m1_len)
    plan.sc_total = sum(plan.slot_len)
    return plan


# ----------------------------------------------------------------------------
# Table building
# ----------------------------------------------------------------------------

PHI = None


def get_phi():
    global PHI
    if PHI is None:
        p = np.arange(128, dtype=np.float64) - 64.0
        y2 = p * p
        y2hi = q11(y2)
        PHI = np.stack([y2hi, y2 - y2hi, p, p,
                        np.ones(128), np.ones(128)])
    return PHI


def _psi_col(psi, col, syp, dx):
    sq = q11(syp)
    srq = q11(syp - sq)
    S = sq + srq
    c = S * S + dx * dx
    c1 = q11(c)
    c2 = q11(c - c1)
    psi[0, col] = 1.0
    psi[1, col] = 1.0
    psi[2, col] = -2.0 * sq
    psi[3, col] = -2.0 * srq
    psi[4, col] = c1
    psi[5, col] = c2


def build_core_tables(plan, core):
    """psi [6, 128 + tot_cols] f32 and pars [128, 8] f32 for one core.

    The matmul computes T = phi^T @ psi in fp32r (inputs truncated to
    ~12 bits); every entry is q11-built so products are exact in fp32
    accumulation and T = (y' - S)^2 + dx^2 for the q11-displaced sample
    S (displacement <= ~1e-5 px):
      phi = [q11(y'^2), y'^2 - q11(y'^2), y', y', 1, 1]   (y' = p - 64)
      psi = [1, 1, -2*sq, -2*srq, c1, c2]
    The 1/w^2 scale and aa/2 exponent ride in pars (per-slot Act args).

    psi columns: [phi | reduce chunks (slot-major) | m1 cols s0,s1,s2]
    pars columns: [aa/2 (slots 0-2), 1/w^2 (3-5), ln bias 1e-12 (6), 0 (7)]
    """
    psi = np.zeros((6, 128 + plan.tot_cols), dtype=np.float64)
    psi[:, :128] = get_phi()
    pars = np.zeros((128, 8), dtype=np.float32)
    pars[:, 6] = 1e-12
    # default every table column to the pad sample (covers chunk pad
    # cols, group padding and absent ranks)
    for col in range(128, 128 + plan.tot_cols):
        _psi_col(psi, col, PAD_SY, 0.0)
    ch_off = np.cumsum([0] + plan.chunks).tolist()
    for s in range(SLOTS):
        piece = plan.slots[s][core]
        pars[:, s] = float(plan.aas[piece.curve]) / 2.0
        pars[:, 3 + s] = 1.0 / (float(plan.widths[piece.curve]) ** 2)
    for (ci, ccol, s, rank0, g, M) in plan.reduces:
        piece = plan.slots[s][core]
        for j in range(g):
            job = (piece.jobs[rank0 + j]
                   if rank0 + j < len(piece.jobs) else None)
            if job is None:
                continue
            base = 128 + ch_off[ci] + ccol + j * M
            for m in range(min(M, len(job.rows))):
                sy, sx = job.rows[m]
                _psi_col(psi, base + m, sy - (job.ytile * 128 + 64.0),
                         job.x - sx)
    m1_base = 128 + plan.chunk_cols
    m1_off = np.cumsum([0] + plan.m1_len).tolist()
    for s in range(SLOTS):
        piece = plan.slots[s][core]
        for k, job in enumerate(piece.m1):
            sy, sx = job.rows[0]
            _psi_col(psi, m1_base + m1_off[s] + k,
                     sy - (job.ytile * 128 + 64.0), job.x - sx)
    return psi.astype(np.float32), pars


def make_in_maps(plan):
    in_maps = []
    for core in range(N_CORES):
        psi, pars = build_core_tables(plan, core)
        in_maps.append({"psi": psi, "pars": pars})
    return in_maps


# ----------------------------------------------------------------------------
# Bass device program
# ----------------------------------------------------------------------------

def build_bass(plan):
    import concourse.bacc as bacc
    import concourse.mybir as mybir
    from concourse.tile import TileContext

    dt = mybir.dt

    class _Bacc(bacc.Bacc):
        """Force Ln/Exp/Relu activations onto the single table set that
        contains all three, so the kernel pays exactly one ACT_TABLE_LOAD."""

        def insert_act_table_loads(self):
            from concourse.hw_specs import get_activation_tables
            mine = {mybir.ActivationFunctionType.Ln,
                    mybir.ActivationFunctionType.Exp,
                    mybir.ActivationFunctionType.Relu}
            all_tables = get_activation_tables(self.m.arch)
            combined = "natural_log_exp_and_others"
            if combined not in all_tables or \
                    not mine <= all_tables[combined]:
                return super().insert_act_table_loads()
            tables = []
            for name, funcs in all_tables.items():
                if name != combined:
                    funcs = funcs - mine
                tables.append((name, funcs))
            bacc._bass_rust.insert_act_table_loads(self, tables)

    nc = _Bacc(None, target_bir_lowering=False)

    SC = plan.sc_total
    psi_d = nc.dram_tensor("psi", [6, 128 + plan.tot_cols], dt.float32r,
                           kind="ExternalInput")
    pars_d = nc.dram_tensor("pars", [128, 8], dt.float32,
                            kind="ExternalInput")
    out_d = nc.dram_tensor("out", [128, SC], dt.bfloat16,
                           kind="ExternalOutput")

    with TileContext(nc) as tc:
        with tc.tile_pool(name="sb", bufs=1) as pool, \
             tc.tile_pool(name="ps", bufs=1, space="PSUM") as ppool:
            psi_t = pool.tile([6, 128 + plan.tot_cols], dt.float32r,
                              tag="psi")
            nc.sync.dma_start(out=psi_t[:], in_=psi_d[:],
                              max_dma_last_dim=512)
            pars_t = pool.tile([128, 8], dt.float32, tag="pars")
            nc.sync.dma_start(out=pars_t[:], in_=pars_d[:])
            phi = psi_t[:, 0:128]

            strips = [ppool.tile([128, plan.slot_len[s]], dt.float32,
                                 tag=f"strip{s}", name=f"strip{s}")
                      for s in range(SLOTS)]
            tail = pool.tile([128, SC], dt.bfloat16, tag="tail")
            s_off = np.cumsum([0] + plan.slot_len).tolist()

            # m1 matmuls write single-sample distances straight into the
            # strip head; chunk matmuls feed the grouped reduces.  PE
            # order interleaves them so slot 0's strip completes first.
            m1_base = 128 + plan.chunk_cols
            m1_off = np.cumsum([0] + plan.m1_len).tolist()

            def emit_m1(s):
                if plan.m1_len[s]:
                    nc.tensor.matmul(
                        strips[s][:, 0:plan.m1_len[s]], phi,
                        psi_t[:, m1_base + m1_off[s]:
                              m1_base + m1_off[s + 1]],
                        start=True, stop=True)

            Ts = [None] * len(plan.chunks)
            col_off = np.cumsum([0] + plan.chunks).tolist()

            def emit_chunk(ci):
                span = plan.chunks[ci]
                Tc = ppool.tile([128, span], dt.float32, tag=f"T{ci}",
                                name=f"T{ci}")
                nc.tensor.matmul(Tc[:], phi,
                                 psi_t[:, 128 + col_off[ci]:
                                       128 + col_off[ci] + span],
                                 start=True, stop=True)
                Ts[ci] = Tc

            emit_chunk(0)
            emit_m1(0)
            for ci in range(1, len(plan.chunks)):
                emit_chunk(ci)
                if ci < SLOTS:
                    emit_m1(ci)
            for s in range(len(plan.chunks), SLOTS):
                emit_m1(s)
            for (ci, ccol, s, rank, g, M) in plan.reduces:
                ov = strips[s][:, plan.m1_len[s] + rank:
                               plan.m1_len[s] + rank + g]
                if M == 2:
                    tv = Ts[ci][:, ccol:ccol + 2 * g].rearrange(
                        "p (j m) -> p j m", j=g, m=2)
                    nc.vector.tensor_tensor(ov, tv[:, :, 0], tv[:, :, 1],
                                            op=mybir.AluOpType.min)
                else:
                    tv = Ts[ci][:, ccol:ccol + g * M].rearrange(
                        "p (j m) -> p j m", j=g, m=M)
                    nc.vector.tensor_reduce(out=ov, in_=tv,
                                            axis=mybir.AxisListType.X,
                                            op=mybir.AluOpType.min)

            # tail per slot: relu -> ln(x/w^2) -> exp(aa/2 * .) -> bf16;
            # strip stays in PSUM, output DMAs ride sync / scalar.
            for s in range(SLOTS):
                st = strips[s][:]
                nc.scalar.activation(st, st,
                                     mybir.ActivationFunctionType.Relu,
                                     bias=pars_t[:, 7:8], scale=1.0)
                nc.scalar.activation(st, st,
                                     mybir.ActivationFunctionType.Ln,
                                     bias=pars_t[:, 6:7],
                                     scale=pars_t[:, 3 + s:4 + s])
                tl = tail[:, s_off[s]:s_off[s + 1]]
                nc.scalar.activation(tl, st,
                                     mybir.ActivationFunctionType.Exp,
                                     bias=pars_t[:, 7:8],
                                     scale=pars_t[:, s:s + 1])
                eng = nc.scalar if s == SLOTS - 1 else nc.sync
                eng.dma_start(out=out_d[:, s_off[s]:s_off[s + 1]], in_=tl)
    nc.compile()
    return nc


# ----------------------------------------------------------------------------
# Host gather/unshard
# ----------------------------------------------------------------------------

def scatter_all(plan, results):
    B = len(plan.widths)
    out = np.zeros((B, H, W), dtype=np.float32)
    s_off = np.cumsum([0] + plan.slot_len).tolist()
    # min-merge p over (curve, ytile, x) -- split jobs contribute twice
    acc = {}
    for core in range(N_CORES):
        p = np.asarray(results[core]["out"]).astype(np.float32)
        for s in range(SLOTS):
            piece = plan.slots[s][core]
            base = s_off[s]
            for k, j in enumerate(piece.m1):
                key = (piece.curve, j.ytile, j.x)
                v = p[:, base + k]
                o = acc.get(key)
                acc[key] = v if o is None else np.minimum(o, v)
            base += plan.m1_len[s]
            for k, j in enumerate(piece.jobs):
                key = (piece.curve, j.ytile, j.x)
                v = p[:, base + k]
                o = acc.get(key)
                acc[key] = v if o is None else np.minimum(o, v)
    for (c, yt, x), v in acc.items():
        out[c, yt * 128:(yt + 1) * 128, x] = np.clip(1.0 - v, 0.0, 1.0)
    return out


# ----------------------------------------------------------------------------
# Host simulation (validation without hardware)
# ----------------------------------------------------------------------------

def simulate_core(plan, core):
    psi, pars = build_core_tables(plan, core)
    phi = psi[:, :128].astype(np.float32)
    T = (phi.T @ psi[:, 128:]).astype(np.float32)
    SC = plan.sc_total
    strip = np.zeros((128, SC), dtype=np.float32)
    s_off = np.cumsum([0] + plan.slot_len).tolist()
    ch_off = np.cumsum([0] + plan.chunks).tolist()
    m1_off = np.cumsum([0] + plan.m1_len).tolist()
    for s in range(SLOTS):
        strip[:, s_off[s]:s_off[s] + plan.m1_len[s]] = \
            T[:, plan.chunk_cols + m1_off[s]:plan.chunk_cols + m1_off[s + 1]]
    for (ci, ccol, s, rank, g, M) in plan.reduces:
        blk = T[:, ch_off[ci] + ccol:ch_off[ci] + ccol + g * M]
        strip[:, s_off[s] + plan.m1_len[s] + rank:
              s_off[s] + plan.m1_len[s] + rank + g] = \
            blk.reshape(128, g, M).min(axis=2)
    tail = np.zeros((128, SC), dtype=np.float32)
    for s in range(SLOTS):
        st = np.maximum(strip[:, s_off[s]:s_off[s + 1]], 0.0)
        ln = np.log(st * pars[0, 3 + s] + 1e-12).astype(np.float32)
        tail[:, s_off[s]:s_off[s + 1]] = np.exp(pars[0, s] * ln)
    import ml_dtypes
    return tail.astype(ml_dtypes.bfloat16)


def simulate_all(plan):
    return [{"out": simulate_core(plan, core)} for core in range(N_CORES)]


# ----------------------------------------------------------------------------
# Entry point
# ----------------------------------------------------------------------------

def kernel(inputs, widths, aa_factors):
    inputs = np.asarray(inputs, dtype=np.float32)
    widths = np.asarray(widths, dtype=np.float32)
    aa_factors = np.asarray(aa_factors, dtype=np.float32)
    plan = plan_all(inputs, widths, aa_factors)
    nc = build_bass(plan)
    from concourse.bass_utils import run_bass_kernel_spmd
    res = run_bass_kernel_spmd(nc, make_in_maps(plan),
                               core_ids=list(range(N_CORES)))
    return scatter_all(plan, res.results)


# revision 4
# speedup vs baseline: 1.0821x; 1.0821x over previous
"""Trainium2 Bass kernel for nn_CurveGraphic2d (retrieval_knn), v2.

Computes, for B=16 cubic Bezier curves, a 256x256 canvas per curve:
    canvas = clip(1 - (min_dist_to_32_samples / w + eps)^aa, 0, 1)

v2 strategy (job-pool sharding, 3 curve-pieces per core):
  * Host: evaluate the 32 samples per curve; emit one "job" per active
    pixel column x and y-tile: the samples relevant to that column
    (|sx - x| <= margin, margin = w + 0.6 -- pixels farther than w
    render 0, so wide fixed margins are wasted work).  Jobs with more
    than M_CAP samples split into sub-jobs (host merges with min).
  * Curves split into 24 round-robin pieces (3 slots x 8 cores), more
    pieces for heavier curves; pieces of similar profile share a slot,
    so the rank-wise-max slot schedule pads little.
  * Device: one DMA brings phi+psi tables; fp32r matmuls phi^T @ psi
    produce T[p, col] = squared distance from pixel row p to the col's
    sample.  Single-sample columns are written by the matmul directly
    into the strip (no reduction needed); multi-sample jobs go through
    grouped strided tensor_reduce mins (tensor_tensor min for M=2).
    Per slot the ACT engine runs Relu -> Ln(x/w^2) -> Exp(aa/2 * .) in
    PSUM and emits bf16; DMAs stream the three slot strips out.
  * Host: p -> clip(1 - p, 0, 1) and scatter/min-merge columns into the
    canvases (placement + unshard).
"""

import math

import numpy as np

H, W = 256, 256
NUM_SAMPLES = 32
MAX_LENGTH = 300.0
EPSILON = 1e-6
N_CORES = 8
SLOTS = 3
MARGIN_PAD = 0.6
PAD_SY = 1500.0
CHUNK_CAP = 512
M_CAP = 4

# DVE cost model for the grouping DP (ns)
RED_FIXED = 250.0
RED_PER_EL = 1.04


# ----------------------------------------------------------------------------
# Host-side geometry (mirrors reference.py in float64)
# ----------------------------------------------------------------------------

def _bezier_eval(cp, ts):
    K = cp.shape[0]
    n = K - 1
    i = np.arange(K)
    binom = np.array([math.comb(n, k) for k in range(K)], dtype=np.float64)
    t = ts[:, None]
    basis = binom * (t ** i) * ((1.0 - t) ** (n - i))
    return basis @ cp


def _decasteljau_left(cp, t):
    pts = cp.copy()
    left = [cp[0]]
    for _ in range(cp.shape[0] - 1):
        pts = (1.0 - t) * pts[:-1] + t * pts[1:]
        left.append(pts[0])
    return np.stack(left)


def compute_samples(inputs):
    """[B, K, 2] normalized control points -> [B, S, 2] sample points (y, x)."""
    ts = np.linspace(0.0, 1.0, NUM_SAMPLES)
    out = []
    for b in range(inputs.shape[0]):
        cp = inputs[b].astype(np.float64) * np.array([H, W], dtype=np.float64)
        approx = _bezier_eval(cp, ts)
        seg = np.diff(approx, axis=0)
        arc = np.sqrt((seg ** 2).sum(-1)).sum()
        t_tr = min(1.0, MAX_LENGTH / (arc + EPSILON))
        out.append(_bezier_eval(_decasteljau_left(cp, t_tr), ts))
    return np.stack(out)  # [B, S, 2] float64


def q11(x):
    """Round to 11 significant bits (safely exact under fp32r's ~12-bit
    input truncation)."""
    x = np.asarray(x, dtype=np.float64)
    m, e = np.frexp(x)
    return np.ldexp(np.round(m * 2048.0), e - 11)


# ----------------------------------------------------------------------------
# Planner
# ----------------------------------------------------------------------------

class Job:
    __slots__ = ("x", "ytile", "rows")

    def __init__(self, x, ytile, rows):
        self.x = x          # pixel column
        self.ytile = ytile  # 0 or 1
        self.rows = rows    # [(sy, sx), ...] float64


def plan_curve(samples, margin):
    """samples [S, 2] (y, x) -> list of Job (single-column windows),
    jobs larger than M_CAP split into balanced sub-jobs."""
    sy = samples[:, 0]
    sx = samples[:, 1]
    lo = np.maximum(np.floor(sx - margin).astype(int), 0)
    hi = np.minimum(np.ceil(sx + margin).astype(int), W - 1)
    active = np.zeros(W, dtype=bool)
    for a, b in zip(lo, hi):
        if a <= b:
            active[a:b + 1] = True
    xs = np.nonzero(active)[0]
    jobs = []
    for x in xs:
        selx = np.abs(sx - x) <= margin
        for yt in (0, 1):
            y0, y1 = yt * 128, yt * 128 + 128
            sely = (sy + margin >= y0) & (sy - margin < y1)
            sel = selx & sely
            n = int(sel.sum())
            if n == 0:
                continue
            rows = list(zip(sy[sel], sx[sel]))
            parts = -(-n // M_CAP)
            for i in range(parts):
                jobs.append(Job(int(x), yt, rows[i::parts]))
    return jobs


class Piece:
    __slots__ = ("curve", "jobs", "m1")

    def __init__(self, curve, jobs):
        # multi-sample jobs (desc by size) and single-sample jobs
        self.jobs = [j for j in jobs if len(j.rows) > 1]
        self.m1 = [j for j in jobs if len(j.rows) == 1]
        self.curve = curve


def make_pieces(all_jobs):
    """all_jobs: per-curve job list -> [SLOTS][N_CORES] pieces.

    Split curve c into k_c round-robin pieces (identical rank profiles),
    k_c proportional to its load; pack pieces into slots longest-profile
    first so each slot's octet holds pieces of similar length."""
    ncell = SLOTS * N_CORES
    sorted_jobs = [sorted(jl, key=lambda j: len(j.rows), reverse=True)
                   for jl in all_jobs]
    loads = [sum(len(j.rows) for j in jl) for jl in sorted_jobs]
    target = sum(loads) / ncell
    k = [max(1, min(N_CORES, int(round(L / target)))) for L in loads]
    while sum(k) > ncell:
        i = min((i for i in range(len(k)) if k[i] > 1),
                key=lambda i: loads[i] / (k[i] - 1))
        k[i] -= 1
    while sum(k) < ncell:
        i = max((i for i in range(len(k)) if k[i] < N_CORES),
                key=lambda i: loads[i] / (k[i] + 1))
        k[i] += 1
    groups = []
    for c, jl in enumerate(sorted_jobs):
        ps = [Piece(c, jl[i::k[c]]) for i in range(k[c])]
        groups.append((len(ps[0].jobs), ps))
    # shortest profiles first: slot 0 small so its tail chain starts
    # earliest, larger slots follow while the ACT engine stays busy
    groups.sort(key=lambda t: t[0])
    flat = [p for _, ps in groups for p in ps]
    return [flat[s * N_CORES:(s + 1) * N_CORES] for s in range(SLOTS)]


def slot_schedule(pieces):
    """Rank-wise max of the pieces' descending multi-job-size lists."""
    ls = [[len(j.rows) for j in p.jobs] for p in pieces]
    n = max((len(x) for x in ls), default=0)
    return [max(2, max((x[i] if i < len(x) else 0) for x in ls))
            for i in range(n)]


def opt_groups(sched):
    """DP: partition the desc-sorted schedule into groups, each padded to
    its max M, minimizing RED_FIXED per group + RED_PER_EL per element."""
    arr = sorted(sched, reverse=True)
    N = len(arr)
    from functools import lru_cache

    @lru_cache(None)
    def dp(i):
        if i >= N:
            return (0.0, ())
        best = (1e30, ())
        top = arr[i]
        for j in range(i + 1, N + 1):
            cost = RED_FIXED + RED_PER_EL * top * (j - i)
            rest, parts = dp(j)
            if cost + rest < best[0]:
                best = (cost + rest, ((j - i, top),) + parts)
        return best

    return list(dp(0)[1])  # [(count, M)]


def pack_chunks(slot_groups):
    """Pack the slots' reduce groups (slot order) into <=CHUNK_CAP-col
    PSUM chunks; groups may split at chunk boundaries.  Chunk spans are
    rounded up to even (fp32r matmul requires even moving/dst widths).
    Returns (chunks, reduces):
      chunks:  [total cols per chunk, even]
      reduces: [(chunk_idx, chunk_col, slot, red_rank, g, M)]
    """
    chunks = []
    reduces = []
    cur = 0
    ranks = [0] * SLOTS
    for s, groups in enumerate(slot_groups):
        for g, M in groups:
            while g > 0:
                if not chunks or cur + M > CHUNK_CAP - 1:
                    chunks.append(0)
                    cur = 0
                take = min(g, (CHUNK_CAP - 1 - cur) // M)
                reduces.append((len(chunks) - 1, cur, s, ranks[s], take, M))
                cur += take * M
                chunks[-1] = cur + (cur % 2)  # pad col if odd
                ranks[s] += take
                g -= take
    return chunks, reduces


class Plan:
    pass


def plan_all(inputs, widths, aas):
    B = inputs.shape[0]
    samples = compute_samples(inputs)
    all_jobs = [plan_curve(samples[b], float(widths[b]) + MARGIN_PAD)
                for b in range(B)]
    slots = make_pieces(all_jobs)          # [SLOTS][N_CORES] pieces
    scheds = [slot_schedule(slots[s]) for s in range(SLOTS)]
    groups = [opt_groups(scheds[s]) for s in range(SLOTS)]
    rank_m = [[m for g, m in groups[s] for _ in range(g)]
              for s in range(SLOTS)]
    chunks, reduces = pack_chunks(groups)
    plan = Plan()
    plan.samples = samples
    plan.widths = widths
    plan.aas = aas
    plan.slots = slots
    plan.scheds = scheds
    plan.groups = groups
    plan.rank_m = rank_m
    plan.chunks = chunks
    plan.reduces = reduces
    # even widths: fp32r matmul requires even moving/dst column counts
    plan.m1_len = [-(-max(len(p.m1) for p in slots[s]) // 2) * 2
                   for s in range(SLOTS)]
    plan.red_len = [len(rank_m[s]) for s in range(SLOTS)]
    plan.slot_len = [plan.m1_len[s] + plan.red_len[s] for s in range(SLOTS)]
    plan.chunk_cols = sum(chunks)
    plan.tot_cols = plan.chunk_cols + sum(plan.m1_len)
    plan.sc_total = sum(plan.slot_len)
    return plan


# ----------------------------------------------------------------------------
# Table building
# ----------------------------------------------------------------------------

PHI = None


def get_phi():
    global PHI
    if PHI is None:
        p = np.arange(128, dtype=np.float64) - 64.0
        y2 = p * p
        y2hi = q11(y2)
        PHI = np.stack([y2hi, y2 - y2hi, p, p,
                        np.ones(128), np.ones(128)])
    return PHI


def _psi_col(psi, col, syp, dx):
    sq = q11(syp)
    srq = q11(syp - sq)
    S = sq + srq
    c = S * S + dx * dx
    c1 = q11(c)
    c2 = q11(c - c1)
    psi[0, col] = 1.0
    psi[1, col] = 1.0
    psi[2, col] = -2.0 * sq
    psi[3, col] = -2.0 * srq
    psi[4, col] = c1
    psi[5, col] = c2


def build_core_tables(plan, core):
    """psi [6, 128 + tot_cols] f32 and pars [128, 8] f32 for one core.

    The matmul computes T = phi^T @ psi in fp32r (inputs truncated to
    ~12 bits); every entry is q11-built so products are exact in fp32
    accumulation and T = (y' - S)^2 + dx^2 for the q11-displaced sample
    S (displacement <= ~1e-5 px):
      phi = [q11(y'^2), y'^2 - q11(y'^2), y', y', 1, 1]   (y' = p - 64)
      psi = [1, 1, -2*sq, -2*srq, c1, c2]
    The 1/w^2 scale and aa/2 exponent ride in pars (per-slot Act args).

    psi columns: [phi | reduce chunks (slot-major) | m1 cols s0,s1,s2]
    pars columns: [aa/2 (slots 0-2), 1/w^2 (3-5), ln bias 1e-12 (6), 0 (7)]
    """
    psi = np.zeros((6, 128 + plan.tot_cols), dtype=np.float64)
    psi[:, :128] = get_phi()
    pars = np.zeros((128, 8), dtype=np.float32)
    pars[:, 6] = 1e-12
    # default every table column to the pad sample (covers chunk pad
    # cols, group padding and absent ranks)
    for col in range(128, 128 + plan.tot_cols):
        _psi_col(psi, col, PAD_SY, 0.0)
    ch_off = np.cumsum([0] + plan.chunks).tolist()
    for s in range(SLOTS):
        piece = plan.slots[s][core]
        pars[:, s] = float(plan.aas[piece.curve]) / 2.0
        pars[:, 3 + s] = 1.0 / (float(plan.widths[piece.curve]) ** 2)
    for (ci, ccol, s, rank0, g, M) in plan.reduces:
        piece = plan.slots[s][core]
        for j in range(g):
            job = (piece.jobs[rank0 + j]
                   if rank0 + j < len(piece.jobs) else None)
            if job is None:
                continue
            base = 128 + ch_off[ci] + ccol + j * M
            for m in range(min(M, len(job.rows))):
                sy, sx = job.rows[m]
                _psi_col(psi, base + m, sy - (job.ytile * 128 + 64.0),
                         job.x - sx)
    m1_base = 128 + plan.chunk_cols
    m1_off = np.cumsum([0] + plan.m1_len).tolist()
    for s in range(SLOTS):
        piece = plan.slots[s][core]
        for k, job in enumerate(piece.m1):
            sy, sx = job.rows[0]
            _psi_col(psi, m1_base + m1_off[s] + k,
                     sy - (job.ytile * 128 + 64.0), job.x - sx)
    return psi.astype(np.float32), pars


def make_in_maps(plan):
    in_maps = []
    for core in range(N_CORES):
        psi, pars = build_core_tables(plan, core)
        in_maps.append({"psi": psi, "pars": pars})
    return in_maps


# ----------------------------------------------------------------------------
# Bass device program
# ----------------------------------------------------------------------------

def build_bass(plan):
    import concourse.bacc as bacc
    import concourse.mybir as mybir
    from concourse.tile import TileContext

    dt = mybir.dt

    class _Bacc(bacc.Bacc):
        """Force Ln/Exp/Relu activations onto the single table set that
        contains all three, so the kernel pays exactly one ACT_TABLE_LOAD."""

        def insert_act_table_loads(self):
            from concourse.hw_specs import get_activation_tables
            mine = {mybir.ActivationFunctionType.Ln,
                    mybir.ActivationFunctionType.Exp,
                    mybir.ActivationFunctionType.Relu}
            all_tables = get_activation_tables(self.m.arch)
            combined = "natural_log_exp_and_others"
            if combined not in all_tables or \
                    not mine <= all_tables[combined]:
                return super().insert_act_table_loads()
            tables = []
            for name, funcs in all_tables.items():
                if name != combined:
                    funcs = funcs - mine
                tables.append((name, funcs))
            bacc._bass_rust.insert_act_table_loads(self, tables)

    nc = _Bacc(None, target_bir_lowering=False)

    SC = plan.sc_total
    psi_d = nc.dram_tensor("psi", [6, 128 + plan.tot_cols], dt.float32r,
                           kind="ExternalInput")
    pars_d = nc.dram_tensor("pars", [128, 8], dt.float32,
                            kind="ExternalInput")
    out_d = nc.dram_tensor("out", [128, SC], dt.bfloat16,
                           kind="ExternalOutput")

    with TileContext(nc) as tc:
        with tc.tile_pool(name="sb", bufs=1) as pool, \
             tc.tile_pool(name="ps", bufs=1, space="PSUM") as ppool:
            psi_t = pool.tile([6, 128 + plan.tot_cols], dt.float32r,
                              tag="psi")
            nc.sync.dma_start(out=psi_t[:], in_=psi_d[:])
            pars_t = pool.tile([128, 8], dt.float32, tag="pars")
            nc.sync.dma_start(out=pars_t[:], in_=pars_d[:])
            phi = psi_t[:, 0:128]

            # warm-up: a dependency-free activation at the head of the
            # scalar queue so the ACT table load runs at t0 instead of
            # gating the first real chain
            warm = pool.tile([128, 8], dt.float32, tag="warm")
            nc.gpsimd.memset(warm[:], 1.0)
            warm2 = pool.tile([128, 8], dt.float32, tag="warm2")
            nc.scalar.activation(warm2[:], warm[:],
                                 mybir.ActivationFunctionType.Ln,
                                 bias=warm[:, 0:1], scale=1.0)

            strips = [ppool.tile([128, plan.slot_len[s]], dt.float32,
                                 tag=f"strip{s}", name=f"strip{s}")
                      for s in range(SLOTS)]
            tail = pool.tile([128, SC], dt.bfloat16, tag="tail")
            s_off = np.cumsum([0] + plan.slot_len).tolist()

            # m1 matmuls write single-sample distances straight into the
            # strip head; chunk matmuls feed the grouped reduces.  PE
            # order interleaves them so slot 0's strip completes first.
            m1_base = 128 + plan.chunk_cols
            m1_off = np.cumsum([0] + plan.m1_len).tolist()

            def emit_m1(s):
                if plan.m1_len[s]:
                    nc.tensor.matmul(
                        strips[s][:, 0:plan.m1_len[s]], phi,
                        psi_t[:, m1_base + m1_off[s]:
                              m1_base + m1_off[s + 1]],
                        start=True, stop=True)

            Ts = [None] * len(plan.chunks)
            col_off = np.cumsum([0] + plan.chunks).tolist()

            def emit_chunk(ci):
                span = plan.chunks[ci]
                Tc = ppool.tile([128, span], dt.float32, tag=f"T{ci}",
                                name=f"T{ci}")
                nc.tensor.matmul(Tc[:], phi,
                                 psi_t[:, 128 + col_off[ci]:
                                       128 + col_off[ci] + span],
                                 start=True, stop=True)
                Ts[ci] = Tc

            emit_chunk(0)
            emit_m1(0)
            for ci in range(1, len(plan.chunks)):
                emit_chunk(ci)
                if ci < SLOTS:
                    emit_m1(ci)
            for s in range(len(plan.chunks), SLOTS):
                emit_m1(s)
            for (ci, ccol, s, rank, g, M) in plan.reduces:
                ov = strips[s][:, plan.m1_len[s] + rank:
                               plan.m1_len[s] + rank + g]
                if M == 2:
                    tv = Ts[ci][:, ccol:ccol + 2 * g].rearrange(
                        "p (j m) -> p j m", j=g, m=2)
                    nc.vector.tensor_tensor(ov, tv[:, :, 0], tv[:, :, 1],
                                            op=mybir.AluOpType.min)
                else:
                    tv = Ts[ci][:, ccol:ccol + g * M].rearrange(
                        "p (j m) -> p j m", j=g, m=M)
                    nc.vector.tensor_reduce(out=ov, in_=tv,
                                            axis=mybir.AxisListType.X,
                                            op=mybir.AluOpType.min)

            # tail per slot: ln(x/w^2) -> exp(aa/2 * .) -> bf16; negative
            # x (fp32r rounding near the curve) makes Ln emit NaN, which
            # the host scatter maps to canvas=1 -- the correct value
            # there.  Strip stays in PSUM; output DMAs ride sync/scalar.
            for s in range(SLOTS):
                st = strips[s][:]
                nc.scalar.activation(st, st,
                                     mybir.ActivationFunctionType.Ln,
                                     bias=pars_t[:, 6:7],
                                     scale=pars_t[:, 3 + s:4 + s])
                tl = tail[:, s_off[s]:s_off[s + 1]]
                nc.scalar.activation(tl, st,
                                     mybir.ActivationFunctionType.Exp,
                                     bias=pars_t[:, 7:8],
                                     scale=pars_t[:, s:s + 1])
                eng = nc.scalar if s == SLOTS - 1 else nc.sync
                eng.dma_start(out=out_d[:, s_off[s]:s_off[s + 1]], in_=tl)
    nc.compile()
    return nc


# ----------------------------------------------------------------------------
# Host gather/unshard
# ----------------------------------------------------------------------------

def scatter_all(plan, results):
    B = len(plan.widths)
    out = np.zeros((B, H, W), dtype=np.float32)
    s_off = np.cumsum([0] + plan.slot_len).tolist()
    # min-merge p over (curve, ytile, x) -- split jobs contribute twice
    acc = {}
    for core in range(N_CORES):
        p = np.asarray(results[core]["out"]).astype(np.float32)
        for s in range(SLOTS):
            piece = plan.slots[s][core]
            base = s_off[s]
            for k, j in enumerate(piece.m1):
                key = (piece.curve, j.ytile, j.x)
                v = p[:, base + k]
                o = acc.get(key)
                acc[key] = v if o is None else np.minimum(o, v)
            base += plan.m1_len[s]
            for k, j in enumerate(piece.jobs):
                key = (piece.curve, j.ytile, j.x)
                v = p[:, base + k]
                o = acc.get(key)
                acc[key] = v if o is None else np.minimum(o, v)
    for (c, yt, x), v in acc.items():
        out[c, yt * 128:(yt + 1) * 128, x] = \
            np.clip(1.0 - np.nan_to_num(v, nan=0.0), 0.0, 1.0)
    return out


# ----------------------------------------------------------------------------
# Host simulation (validation without hardware)
# ----------------------------------------------------------------------------

def simulate_core(plan, core):
    psi, pars = build_core_tables(plan, core)
    phi = psi[:, :128].astype(np.float32)
    T = (phi.T @ psi[:, 128:]).astype(np.float32)
    SC = plan.sc_total
    strip = np.zeros((128, SC), dtype=np.float32)
    s_off = np.cumsum([0] + plan.slot_len).tolist()
    ch_off = np.cumsum([0] + plan.chunks).tolist()
    m1_off = np.cumsum([0] + plan.m1_len).tolist()
    for s in range(SLOTS):
        strip[:, s_off[s]:s_off[s] + plan.m1_len[s]] = \
            T[:, plan.chunk_cols + m1_off[s]:plan.chunk_cols + m1_off[s + 1]]
    for (ci, ccol, s, rank, g, M) in plan.reduces:
        blk = T[:, ch_off[ci] + ccol:ch_off[ci] + ccol + g * M]
        strip[:, s_off[s] + plan.m1_len[s] + rank:
              s_off[s] + plan.m1_len[s] + rank + g] = \
            blk.reshape(128, g, M).min(axis=2)
    tail = np.zeros((128, SC), dtype=np.float32)
    with np.errstate(invalid="ignore", divide="ignore"):
        for s in range(SLOTS):
            st = strip[:, s_off[s]:s_off[s + 1]]
            ln = np.log(st * pars[0, 3 + s] + 1e-12).astype(np.float32)
            tail[:, s_off[s]:s_off[s + 1]] = np.exp(pars[0, s] * ln)
    import ml_dtypes
    return tail.astype(ml_dtypes.bfloat16)


def simulate_all(plan):
    return [{"out": simulate_core(plan, core)} for core in range(N_CORES)]


# ----------------------------------------------------------------------------
# Entry point
# ----------------------------------------------------------------------------

def kernel(inputs, widths, aa_factors):
    inputs = np.asarray(inputs, dtype=np.float32)
    widths = np.asarray(widths, dtype=np.float32)
    aa_factors = np.asarray(aa_factors, dtype=np.float32)
    plan = plan_all(inputs, widths, aa_factors)
    nc = build_bass(plan)
    from concourse.bass_utils import run_bass_kernel_spmd
    res = run_bass_kernel_spmd(nc, make_in_maps(plan),
                               core_ids=list(range(N_CORES)))
    return scatter_all(plan, res.results)
